# revision 2
# baseline (speedup 1.0000x reference)
"""Trainium2 Bass kernel for MHA cross-attention (nn_MHACross).

Sharding: 8 cores = 2 batches x 4 head-groups (2 heads each), as v1.

v2 changes vs the 279us baseline:
  * Softmax denominator Z moved off the PE: exp tiles are tree-summed on
    the vector engine in fp16 (DVE 2X mode, ~0.69us per [128,1024] add,
    two parity chains per block), then 4 ones^T matmuls per block reduce
    the chain accumulators.  Replaces 192 PE matmuls with 16.
  * fp16 activations/weights everywhere (same PE rate as bf16, 2X DVE,
    10-bit mantissa keeps the Z tree accumulation accurate).
  * st-granular software pipeline: one unified score-unit stream with the
    attnV stream lagged 6 units behind, and projection / vproj / outproj
    work slotted between units as fillers, so the in-order PE queue never
    sits on a dependency and the scalar engine's 126us exp pipeline
    starts ~25us into the kernel.
  * Column-chunked input DMA across three queues (sync=xmel, gpsimd=
    weights+x+y, vector=rope tables).
  * PSUM: psA 2x[128,1024] (scores/proj/outproj), psB 4x[128,512]
    (attnV accumulators, vproj, Z rows) = exactly 8 banks.
  * Projection inputs (x, xmel, rope tables, Wq/Wk/Wv) live in a pool
    that closes once projections finish, capping SBUF peak.
"""
import sys
sys.path.insert(0, '/opt/trn_rl_repo')
import numpy as np

DIM = 1024
NHEADS = 8
HD = 128          # head dim
HPC = 2           # heads per core
NG = 4            # head groups (cores per batch)
B, T, S = 2, 2048, 3000
NKT = DIM // 128  # contraction tiles
ROPE_BASE = 10000.0
CW = 1024         # pair-chunk width (2 psum banks; matmuls are 512-wide)
LAG = 6           # attnV stream lag behind score stream, in st units

_cache = {}


def _ceil_div(a, b):
    return (a + b - 1) // b


def build_nc(T=T, S=S):
    from concourse import bacc, mybir
    from concourse.tile import TileContext

    f32 = mybir.dt.float32
    f16 = mybir.dt.float16

    nc = bacc.Bacc("TRN2", target_bir_lowering=False, debug=False, num_devices=8)

    xT = nc.dram_tensor("xT", [DIM, T], f16, kind="ExternalInput")
    xmelT = nc.dram_tensor("xmelT", [DIM, S], f16, kind="ExternalInput")
    WqT = nc.dram_tensor("WqT", [128, NKT * HPC * HD], f16, kind="ExternalInput")
    WkT = nc.dram_tensor("WkT", [128, NKT * HPC * HD], f16, kind="ExternalInput")
    WvT = nc.dram_tensor("WvT", [128, NKT * HPC * HD], f16, kind="ExternalInput")
    WoT = nc.dram_tensor("WoT", [HPC * HD, DIM], f16, kind="ExternalInput")
    cosq = nc.dram_tensor("cosq", [HD, T], f16, kind="ExternalInput")
    sinq = nc.dram_tensor("sinq", [HD, T], f16, kind="ExternalInput")
    cosk = nc.dram_tensor("cosk", [HD, S], f16, kind="ExternalInput")
    sink = nc.dram_tensor("sink", [HD, S], f16, kind="ExternalInput")
    y = nc.dram_tensor("y", [T, DIM], f16, kind="ExternalOutput")
    n_pairs = _ceil_div(T, CW)                                  # 2

    n_st = _ceil_div(S, 128)                                    # 24
    k_chunks = [(i * CW, min(CW, S - i * CW)) for i in range(_ceil_div(S, CW))]
    t_pairs = [(i * CW, min(CW, T - i * CW)) for i in range(n_pairs)]
    blocks = [(pi, h) for pi in range(n_pairs) for h in range(HPC)]
    n_units = len(blocks) * n_st                                # 96

    with TileContext(nc) as tc:
        with tc.tile_pool(name="wpool", bufs=1) as wp, \
             tc.tile_pool(name="persist", bufs=1) as pp:
            # ---------- persistent tiles (live the whole kernel) ----------
            wo = []
            for h in range(HPC):
                wo_h = wp.tile([128, DIM], f16, name=f"wo{h}", uniquify=True)
                wo.append(wo_h)
            ones = wp.tile([128, 1], f16)
            nc.vector.memset(ones[:], 1.0)
            ones_k1 = wp.tile([1, 128], f16)
            nc.vector.memset(ones_k1[:], 1.0)

            kT_r = [pp.tile([128, S], f16, name=f"kT{h}", uniquify=True) for h in range(HPC)]
            qT_r = [pp.tile([128, T], f16, name=f"qT{h}", uniquify=True) for h in range(HPC)]
            v_sb = pp.tile([128, n_st, HPC * HD], f16)

            with tc.tile_pool(name="pP", bufs=12) as pP, \
                 tc.tile_pool(name="zaP", bufs=4) as zaP, \
                 tc.tile_pool(name="aoP", bufs=3) as aoP, \
                 tc.tile_pool(name="zbP", bufs=2) as zbP, \
                 tc.tile_pool(name="rtP", bufs=2) as rtp, \
                 tc.tile_pool(name="psA", bufs=2, space="PSUM") as psA, \
                 tc.tile_pool(name="psB", bufs=1, space="PSUM") as psB:

                # ---------- emission helpers ----------
                bstate = {}

                def proj_rope(h, c0, cw, w_sb, src, cos_sb, sin_sb, out_sl):
                    ps = psA.tile([128, CW], f32, name="prps", tag="sc", bufs=2)
                    for kt in range(NKT):
                        for ci in range(_ceil_div(cw, 512)):
                            w0 = ci * 512
                            wn = min(512, cw - w0)
                            nc.tensor.matmul(
                                ps[:, w0:w0 + wn],
                                w_sb[:, kt, h * HD:(h + 1) * HD],
                                src[kt][:, c0 + w0:c0 + w0 + wn],
                                start=(kt == 0), stop=(kt == NKT - 1),
                                skip_group_check=True)
                    swp = rtp.tile([128, CW], f16, name="swp", tag="rt", bufs=2)
                    nc.scalar.copy(swp[0:64, :cw], ps[64:128, :cw])
                    nc.scalar.copy(swp[64:128, :cw], ps[0:64, :cw])
                    nc.vector.tensor_mul(swp[:, :cw], swp[:, :cw], sin_sb[:, c0:c0 + cw])
                    nc.vector.tensor_mul(out_sl, ps[:, :cw], cos_sb[:, c0:c0 + cw])
                    nc.vector.tensor_add(out_sl, out_sl, swp[:, :cw])

                def sc_st(pi, h, st, slot=-1):  # slot kept for schedule clarity
                    c0, cw = t_pairs[pi]
                    bk = bstate.setdefault((pi, h), {"p": {}, "za": [None, None],
                                                     "seed": [None, None]})
                    s0 = st * 128
                    scnt = min(128, S - s0)
                    scps = psA.tile([128, CW], f32, name="scps", tag="sc", bufs=2)
                    for ci in range(_ceil_div(cw, 512)):
                        w0 = ci * 512
                        wn = min(512, cw - w0)
                        nc.tensor.matmul(
                            scps[:scnt, w0:w0 + wn],
                            kT_r[h][:, s0:s0 + scnt],
                            qT_r[h][:, c0 + w0:c0 + w0 + wn],
                            start=True, stop=True,
                            skip_group_check=True)
                    p_t = pP.tile([128, CW], f16, name="p_t", tag="p", bufs=12)
                    nc.scalar.activation(p_t[:scnt, :cw], scps[:scnt, :cw],
                                         mybir.ActivationFunctionType.Exp)
                    bk["p"][st] = (p_t, scnt)
                    # fp16 Z tree accumulation on the vector engine, 2 chains
                    ch = st % 2
                    if bk["za"][ch] is None:
                        if bk["seed"][ch] is None:
                            bk["seed"][ch] = (p_t, scnt)
                        else:
                            sp, ssc = bk["seed"][ch]
                            za = zaP.tile([128, CW], f16, name="za", tag="za", bufs=4)
                            nc.vector.tensor_add(za[:, :cw], sp[:, :cw], p_t[:, :cw])
                            bk["za"][ch] = za
                    else:
                        za = bk["za"][ch]
                        nc.vector.tensor_add(za[:scnt, :cw], za[:scnt, :cw],
                                             p_t[:scnt, :cw])

                def av_st(pi, h, st):
                    c0, cw = t_pairs[pi]
                    bk = bstate[(pi, h)]
                    nci = _ceil_div(cw, 512)
                    if st == 0:
                        bk["o2"] = [psB.tile([128, 512], f32, name="o2ps", tag="o2", bufs=3)
                                    for _ in range(nci)]
                    p_t, scnt = bk["p"].pop(st)
                    for ci in range(nci):
                        w0 = ci * 512
                        wn = min(512, cw - w0)
                        nc.tensor.matmul(
                            bk["o2"][ci][:, :wn],
                            v_sb[:scnt, st, h * HD:(h + 1) * HD],
                            p_t[:scnt, w0:w0 + wn],
                            start=(st == 0), stop=(st == n_st - 1))

                def fin(pi, h):
                    c0, cw = t_pairs[pi]
                    bk = bstate[(pi, h)]
                    nci = _ceil_div(cw, 512)
                    zsb = zbP.tile([1, CW], f16, name="zsb", tag="zsb", bufs=2)
                    for ci in range(nci):
                        w0 = ci * 512
                        wn = min(512, cw - w0)
                        zps = psB.tile([1, 512], f32, name="zps", tag="vz", bufs=1)
                        nc.tensor.matmul(zps[:, :wn], ones[:, :],
                                         bk["za"][0][:, w0:w0 + wn],
                                         start=True, stop=False)
                        nc.tensor.matmul(zps[:, :wn], ones[:, :],
                                         bk["za"][1][:, w0:w0 + wn],
                                         start=False, stop=True)
                        nc.vector.tensor_copy(zsb[:, w0:w0 + wn], zps[:, :wn])
                    zrep_ps = psA.tile([128, CW], f32, name="zrep_ps", tag="sc", bufs=2)
                    for ci in range(nci):
                        w0 = ci * 512
                        wn = min(512, cw - w0)
                        nc.tensor.matmul(zrep_ps[:, w0:w0 + wn], ones_k1[:, :],
                                         zsb[:, w0:w0 + wn], start=True, stop=True,
                                         skip_group_check=True)
                    zr2 = zbP.tile([128, CW], f32, name="zr2", tag="zr2", bufs=1)
                    nc.vector.reciprocal_approx_fast(out=zr2[:, :cw], in_=zrep_ps[:, :cw])
                    ao = aoP.tile([128, CW], f16, name="ao", tag="ao", bufs=3)
                    for ci in range(nci):
                        w0 = ci * 512
                        wn = min(512, cw - w0)
                        nc.vector.tensor_mul(ao[:, w0:w0 + wn], bk["o2"][ci][:, :wn],
                                             zr2[:, w0:w0 + wn])
                    bk["ao"] = ao

                yP_holder = {}

                def outproj_tt(pi, tt):
                    c0, cw = t_pairs[pi]
                    yps = psA.tile([128, CW], f32, name="yps", tag="sc", bufs=2)
                    for nn in range(DIM // 512):
                        for h in range(HPC):
                            nc.tensor.matmul(
                                yps[:, nn * 512:(nn + 1) * 512],
                                bstate[(pi, h)]["ao"][:, tt * 128:(tt + 1) * 128],
                                wo[h][:, nn * 512:(nn + 1) * 512],
                                start=(h == 0), stop=(h == HPC - 1),
                                skip_group_check=True)
                    y_sb = yP_holder["pool"].tile([128, DIM], f16, name="y_sb",
                                                  tag="ysb", bufs=2)
                    if pi == n_pairs - 1 and tt % 2 == 0:
                        nc.scalar.copy(y_sb[:], yps[:])
                    else:
                        nc.vector.tensor_copy(y_sb[:], yps[:])
                    q = nc.gpsimd if tt % 2 == 0 else nc.sync
                    q.dma_start(out=y[c0 + tt * 128: c0 + (tt + 1) * 128, :],
                                in_=y_sb[:])

                # ---------- unified slot schedule ----------
                # sc unit i: block i//24, st i%24. av unit i-LAG. fillers by slot.
                def unit(i):
                    pi, h = blocks[i // n_st]
                    return pi, h, i % n_st

                # ---------- section 1: projections + B0/B1 ----------------
                with tc.tile_pool(name="projP", bufs=1) as jp:
                    wq = jp.tile([128, NKT, HPC * HD], f16)
                    wk = jp.tile([128, NKT, HPC * HD], f16)
                    wv = jp.tile([128, NKT, HPC * HD], f16)
                    csq = jp.tile([128, T], f16, name="csq")
                    snq = jp.tile([128, T], f16, name="snq")
                    csk = jp.tile([128, S], f16, name="csk")
                    snk = jp.tile([128, S], f16, name="snk")
                    xq = [jp.tile([128, T], f16, name=f"xq{kt}", uniquify=True)
                          for kt in range(NKT)]
                    xm = [jp.tile([128, S], f16, name=f"xm{kt}", uniquify=True)
                          for kt in range(NKT)]

                    # input DMA emission (queue order == stream order)
                    nc.scalar.dma_start(out=csk[0:64, :], in_=cosk[0:64, :])
                    nc.scalar.dma_start(out=snk[64:128, :], in_=sink[64:128, :])
                    nc.sync.dma_start(out=wk[:], in_=WkT[:].rearrange("p (k n) -> p k n", k=NKT))
                    for (c0, cw) in k_chunks:
                        for kt in range(NKT):
                            nc.sync.dma_start(out=xm[kt][:, c0:c0 + cw],
                                              in_=xmelT[kt * 128:(kt + 1) * 128, c0:c0 + cw])
                    nc.scalar.dma_start(out=csq[0:64, :], in_=cosq[0:64, :])
                    nc.scalar.dma_start(out=snq[64:128, :], in_=sinq[64:128, :])
                    nc.gpsimd.dma_start(out=wv[:], in_=WvT[:].rearrange("p (k n) -> p k n", k=NKT))
                    c0, cw = t_pairs[0]
                    for kt in range(NKT):
                        nc.gpsimd.dma_start(out=xq[kt][:, c0:c0 + cw],
                                            in_=xT[kt * 128:(kt + 1) * 128, c0:c0 + cw])
                    nc.gpsimd.dma_start(out=wq[:], in_=WqT[:].rearrange("p (k n) -> p k n", k=NKT))
                    c0, cw = t_pairs[1]
                    for kt in range(NKT):
                        nc.gpsimd.dma_start(out=xq[kt][:, c0:c0 + cw],
                                            in_=xT[kt * 128:(kt + 1) * 128, c0:c0 + cw])
                    for h in range(HPC):
                        nc.gpsimd.dma_start(out=wo[h][:], in_=WoT[h * HD:(h + 1) * HD, :])

                    # expand half rope tables on the vector engine
                    nc.vector.tensor_copy(csk[64:128, :], csk[0:64, :])
                    nc.vector.tensor_scalar_mul(snk[0:64, :], snk[64:128, :], -1.0)
                    nc.vector.tensor_copy(csq[64:128, :], csq[0:64, :])
                    nc.vector.tensor_scalar_mul(snq[0:64, :], snq[64:128, :], -1.0)

                    def kproj(h, ci):
                        c0, cw = k_chunks[ci]
                        proj_rope(h, c0, cw, wk, xm, csk, snk, kT_r[h][:, c0:c0 + cw])

                    def qproj(h, pi):
                        c0, cw = t_pairs[pi]
                        proj_rope(h, c0, cw, wq, xq, csq, snq, qT_r[h][:, c0:c0 + cw])

                    def vproj_st(st):
                        s0 = st * 128
                        scnt = min(128, S - s0)
                        vps = psB.tile([128, 512], f32, name="vps", tag="vz", bufs=1)
                        for kt in range(NKT):
                            nc.tensor.matmul(
                                vps[:scnt, :HPC * HD],
                                xm[kt][:, s0:s0 + scnt],
                                wv[:, kt, :],
                                start=(kt == 0), stop=(kt == NKT - 1))
                        nc.vector.tensor_copy(v_sb[:scnt, st, :], vps[:scnt, :HPC * HD])

                    fillers1 = {2: [lambda: kproj(0, 1)],
                                6: [lambda: kproj(1, 0)],
                                10: [lambda: kproj(0, 2)],
                                14: [lambda: kproj(1, 1)],
                                18: [lambda: kproj(1, 2)],
                                22: [lambda: qproj(1, 0)],
                                26: [lambda: qproj(0, 1)],
                                30: [lambda: qproj(1, 1)]}

                    kproj(0, 0)
                    qproj(0, 0)
                    SEC1 = 36
                    for i in range(SEC1):
                        for f in fillers1.get(i, []):
                            f()
                        if i < 18:
                            vproj_st(i)
                        elif 24 <= i < 30:
                            vproj_st(i - 6)
                        sc_st(*unit(i), slot=i)
                        if i >= LAG:
                            pi, h, st = unit(i - LAG)
                            av_st(pi, h, st)
                            if st == n_st - 1:
                                fin(pi, h)

                # ---------- section 2: B2/B3 + out-projection --------------
                with tc.tile_pool(name="yP", bufs=2) as yp:
                    yP_holder["pool"] = yp
                    fillers2 = {}
                    for j, sl in enumerate(range(57, 81, 3)):
                        fillers2.setdefault(sl, []).append(
                            (lambda tt: (lambda: outproj_tt(0, tt)))(j))
                    for i in range(SEC1, n_units + LAG):
                        for f in fillers2.get(i, []):
                            f()
                        if i < n_units:
                            sc_st(*unit(i), slot=i)
                        pi, h, st = unit(i - LAG)
                        av_st(pi, h, st)
                        if st == n_st - 1:
                            fin(pi, h)
                    for tt in range(t_pairs[1][1] // 128):
                        outproj_tt(1, tt)

    nc.compile()
    return nc


def _host_tables(T=T, S=S):
    scale = float(HD) ** (-0.25)
    inv = 1.0 / (ROPE_BASE ** (np.arange(0, HD, 2, dtype=np.float64) / HD))  # [64]

    def tables(L):
        fr = np.outer(inv, np.arange(L, dtype=np.float64))  # [64, L]
        c = np.cos(fr) * scale
        s = np.sin(fr) * scale
        cos = np.concatenate([c, c], axis=0).astype(np.float16)
        sin = np.concatenate([-s, s], axis=0).astype(np.float16)
        return np.ascontiguousarray(cos), np.ascontiguousarray(sin)

    cosq_, sinq_ = tables(T)
    cosk_, sink_ = tables(S)
    return cosq_, sinq_, cosk_, sink_


def make_in_maps(x, xmel, Wq, Wkv, Wout):
    f16 = np.float16
    Bx, Tx, C = x.shape
    Sx = xmel.shape[1]
    cosq_, sinq_, cosk_, sink_ = _host_tables(Tx, Sx)

    x = np.asarray(x, dtype=np.float32)
    xmel = np.asarray(xmel, dtype=np.float32)
    Wq = np.asarray(Wq, dtype=np.float32)
    Wkv = np.asarray(Wkv, dtype=np.float32)
    Wout = np.asarray(Wout, dtype=np.float32)

    xT_b = [np.ascontiguousarray(x[b].T).astype(f16) for b in range(Bx)]
    xmelT_b = [np.ascontiguousarray(xmel[b].T).astype(f16) for b in range(Bx)]
    gsz = HPC * HD  # 256
    WqT_g, WkT_g, WvT_g, WoT_g = [], [], [], []
    for g in range(NG):
        r0 = g * gsz
        def prearr(wt):  # [DIM, gsz] -> [128, NKT*gsz], row p holds [kt, n]
            return np.ascontiguousarray(
                wt.reshape(NKT, 128, gsz).transpose(1, 0, 2).reshape(128, NKT * gsz)).astype(f16)
        WqT_g.append(prearr(Wq[r0:r0 + gsz, :].T))
        WkT_g.append(prearr(Wkv[r0:r0 + gsz, :].T))
        WvT_g.append(prearr(Wkv[DIM + r0:DIM + r0 + gsz, :].T))
        WoT_g.append(np.ascontiguousarray(Wout[:, r0:r0 + gsz].T).astype(f16))

    in_maps = []
    for c in range(Bx * NG):
        b, g = c // NG, c % NG
        in_maps.append({
            "xT": xT_b[b], "xmelT": xmelT_b[b],
            "WqT": WqT_g[g], "WkT": WkT_g[g], "WvT": WvT_g[g], "WoT": WoT_g[g],
            "cosq": cosq_, "sinq": sinq_, "cosk": cosk_, "sink": sink_,
        })
    return in_maps


def kernel(x, xmel, Wq, Wkv, Wout):
    from concourse.bass_utils import run_bass_kernel_spmd

    x = np.asarray(x, dtype=np.float32)
    xmel = np.asarray(xmel, dtype=np.float32)
    Bx, Tx, C = x.shape
    Sx = xmel.shape[1]
    assert (Bx, Tx, C, Sx) == (B, T, DIM, S)

    if "nc" not in _cache:
        _cache["nc"] = build_nc()
    nc = _cache["nc"]

    in_maps = make_in_maps(x, xmel,
                           np.asarray(Wq, dtype=np.float32),
                           np.asarray(Wkv, dtype=np.float32),
                           np.asarray(Wout, dtype=np.float32))
    res = run_bass_kernel_spmd(nc, in_maps, list(range(8)))
    out = np.zeros((B, T, DIM), dtype=np.float32)
    for c in range(8):
        b = c // NG
        out[b] += res.results[c]["y"]
    return out
